# revision 5
# baseline (speedup 1.0000x reference)
"""Dilated attention (DilatedAttentionOP) as a Bass/Tile SPMD kernel on 8 TRN2 NeuronCores.

Reference semantics (hardcoded; see problem reference):
  DIL=(1,2,4), SEG=(512,512,512), H=16, D=1024, B=2, L=8192.
  pairs truncate to [(br0,off0),(br1,off0),(br1,off1)]:
    - branch 0 (dilation 1): standard attention over contiguous 512-token segments,
      weights Wqkv[0]/Wout[0], covers every position.
    - branch 1 (dilation 2): attention over 512-token segments of the even-position
      subsequence (off=0) and odd-position subsequence (off=1), weights index 1.
  output = (out_branch0 + out_branch1_interleaved) / 3  (+ (bout0+bout1)/3).

Sharding: 64 independent (batch, segment) attention problems of 512 tokens each;
cores 0-3 take the 32 branch-0 segments, cores 4-7 the 32 branch-1 segments
(8 segments/core, identical SPMD program; per-branch weights replicated to the
core's in_map).

Per-core math (bf16 matmuls, fp32 PSUM accumulate):
  xT[seg] [D=1024, 512] -> QT,KT [128ch x 512]x16 tiles (channels on partitions,
  Q pre-scaled by 1/8 folded into Wq), V natural [512tok, 1024ch].
  scoresT[k,q] per head via row-paired (tile_position) K=64 matmuls;
  exp on ACT; AVT pair-packed via col-paired matmuls; softmax denominator via
  ones[128,64]-lhsT matmuls giving denom broadcast across partitions; DVE
  reciprocal+multiply; out-projection back to natural [512tok, D] layout
  (Wout pre-scaled by 1/3).
"""

import numpy as np
import ml_dtypes

from concourse import bacc, mybir
from concourse import tile
from concourse.bass_utils import run_bass_kernel_spmd

BF16 = ml_dtypes.bfloat16
F32 = np.float32

# Problem constants (hardcoded per contract)
B, L, D, H = 2, 8192, 1024, 16
HD = D // H            # 64
S = 512                # segment length
NSEG = 8               # segments per core
NCORE = 8
P = 128
DK = D // P            # 8 contraction chunks over D
KT = S // P            # 4 k tiles per segment
NPAIR = H // 2         # 8 head pairs
SCALE_Q = 1.0 / np.sqrt(HD)  # 0.125
W_MIX = 1.0 / 3.0


def build_program(with_bias: bool, repeat: int = 1):
    """repeat>1 re-executes the whole per-core pipeline that many times
    (overwriting the same outputs) — used only for differential timing."""
    nc = bacc.Bacc(None, target_bir_lowering=False)
    dt = mybir.dt
    xt_d = nc.declare_dram_parameter("xt", [NSEG, D, S], dt.bfloat16, False)
    wqkv_d = nc.declare_dram_parameter("wqkv", [D, 3 * D], dt.bfloat16, False)
    wout_d = nc.declare_dram_parameter("wout", [D, D], dt.bfloat16, False)
    if with_bias:
        bq_d = nc.declare_dram_parameter("bqkv", [1, 3 * D], dt.bfloat16, False)
    out_d = nc.declare_dram_parameter("out", [NSEG, S, D], dt.float32, True)

    with tile.TileContext(nc) as tc:
        with (
            tc.tile_pool(name="wpool", bufs=1) as wpool,
            tc.tile_pool(name="xpool", bufs=2) as xpool,
            tc.tile_pool(name="qkpool", bufs=2) as qkpool,
            tc.tile_pool(name="vpool", bufs=2) as vpool,
            tc.tile_pool(name="epool", bufs=6) as epool,
            tc.tile_pool(name="apool", bufs=2) as apool,
            tc.tile_pool(name="rpool", bufs=2) as rpool,
            tc.tile_pool(name="opool", bufs=3) as opool,
            tc.tile_pool(name="pmm", bufs=2, space="PSUM") as pmm,
            tc.tile_pool(name="psc", bufs=2, space="PSUM") as psc,
            tc.tile_pool(name="pav", bufs=2, space="PSUM") as pav,
            tc.tile_pool(name="pdn", bufs=2, space="PSUM") as pdn,
        ):
            # Resident weights
            wqkv_sb = wpool.tile([P, DK, 3 * D], dt.bfloat16)
            nc.sync.dma_start(
                wqkv_sb[:], wqkv_d.rearrange("(dk p) c -> p dk c", p=P)
            )
            wout_sb = wpool.tile([P, DK, D], dt.bfloat16)
            nc.sync.dma_start(
                wout_sb[:], wout_d.rearrange("(dk p) c -> p dk c", p=P)
            )
            ones64 = wpool.tile([P, HD], dt.bfloat16)
            nc.vector.memset(ones64[:], 1.0)
            if with_bias:
                bq_sb = wpool.tile([1, 3 * D], dt.bfloat16)
                nc.sync.dma_start(bq_sb[:], bq_d[:])
                ones_row = wpool.tile([1, S], dt.bfloat16)
                nc.vector.memset(ones_row[:], 1.0)
                ones_col = wpool.tile([1, P], dt.bfloat16)
                nc.vector.memset(ones_col[:], 1.0)

            for s in [s for _ in range(repeat) for s in range(NSEG)]:
                xt_sb = xpool.tile([P, DK, S], dt.bfloat16, tag="xt")
                nc.sync.dma_start(
                    xt_sb[:], xt_d[s].rearrange("(dk p) t -> p dk t", p=P)
                )

                # ---- Stage A: QKV projections ----
                # Q/K in transposed layout: psum[ch_tile(128), tok(512)]
                NQK = 2 * D // P  # 16 tiles: m<8 Q, m>=8 K
                qk_sb = qkpool.tile([P, NQK, S], dt.bfloat16, tag="qk")
                for m in range(NQK):
                    ps = pmm.tile([P, S], dt.float32, tag="mm")
                    for dk in range(DK):
                        nc.tensor.matmul(
                            ps[:],
                            wqkv_sb[:, dk, m * P:(m + 1) * P],
                            xt_sb[:, dk, :],
                            start=(dk == 0),
                            stop=(dk == DK - 1 and not with_bias),
                        )
                    if with_bias:
                        nc.tensor.matmul(
                            ps[:],
                            bq_sb[0:1, m * P:(m + 1) * P],
                            ones_row[0:1, :],
                            start=False,
                            stop=True,
                        )
                    nc.vector.tensor_copy(qk_sb[:, m, :], ps[:])

                # V in natural layout: psum[tok_tile(128), vch(512)]
                v_sb = vpool.tile([P, KT, D], dt.bfloat16, tag="v")
                for tt in range(KT):
                    for nv in range(2):
                        ps = pmm.tile([P, S], dt.float32, tag="mm")
                        for dk in range(DK):
                            nc.tensor.matmul(
                                ps[:],
                                xt_sb[:, dk, tt * P:(tt + 1) * P],
                                wqkv_sb[:, dk, 2 * D + nv * S:2 * D + (nv + 1) * S],
                                start=(dk == 0),
                                stop=(dk == DK - 1 and not with_bias),
                            )
                        if with_bias:
                            nc.tensor.matmul(
                                ps[:],
                                ones_col[0:1, :],
                                bq_sb[0:1, 2 * D + nv * S:2 * D + (nv + 1) * S],
                                start=False,
                                stop=True,
                            )
                        nc.scalar.copy(v_sb[:, tt, nv * S:(nv + 1) * S], ps[:])

                # ---- Stage B: attention per head pair ----
                attn_sb = apool.tile([P, NPAIR, S], dt.bfloat16, tag="attn")
                for pr in range(NPAIR):
                    av = pav.tile([P, S], dt.float32, tag="av")
                    dn = pdn.tile([P, S], dt.float32, tag="dn")
                    for kt in range(KT):
                        # scoresT = K_chunk @ Q^T for the two heads (row-paired)
                        s1 = psc.tile([P, S], dt.float32, tag="sc")
                        s2 = psc.tile([P, S], dt.float32, tag="sc")
                        nc.tensor.matmul(
                            s1[:],
                            qk_sb[0:HD, NPAIR + pr, kt * P:(kt + 1) * P],
                            qk_sb[0:HD, pr, :],
                            start=True, stop=True,
                        )
                        nc.tensor.matmul(
                            s2[:],
                            qk_sb[HD:P, NPAIR + pr, kt * P:(kt + 1) * P],
                            qk_sb[HD:P, pr, :],
                            start=True, stop=True,
                        )
                        e1 = epool.tile([P, S], dt.bfloat16, tag="e")
                        e2 = epool.tile([P, S], dt.bfloat16, tag="e")
                        nc.scalar.activation(
                            e1[:], s1[:], mybir.ActivationFunctionType.Exp
                        )
                        nc.scalar.activation(
                            e2[:], s2[:], mybir.ActivationFunctionType.Exp
                        )
                        # AVT accumulate, col-paired: h1 -> rows 0:64, h2 -> 64:128
                        nc.tensor.matmul(
                            av[0:HD, :],
                            v_sb[:, kt, (2 * pr) * HD:(2 * pr + 1) * HD],
                            e1[:],
                            start=(kt == 0), stop=(kt == KT - 1),
                        )
                        nc.tensor.matmul(
                            av[HD:P, :],
                            v_sb[:, kt, (2 * pr + 1) * HD:(2 * pr + 2) * HD],
                            e2[:],
                            start=(kt == 0), stop=(kt == KT - 1),
                        )
                        # denominators, broadcast over 64 partitions each
                        nc.tensor.matmul(
                            dn[0:HD, :], ones64[:], e1[:],
                            start=(kt == 0), stop=(kt == KT - 1),
                        )
                        nc.tensor.matmul(
                            dn[HD:P, :], ones64[:], e2[:],
                            start=(kt == 0), stop=(kt == KT - 1),
                        )
                    rcp = rpool.tile([P, S], dt.float32, tag="rcp")
                    nc.vector.reciprocal(rcp[:], dn[:])
                    nc.vector.tensor_mul(attn_sb[:, pr, :], av[:], rcp[:])

                # ---- Stage C: output projection (natural layout) ----
                for tt in range(KT):
                    o_sb = opool.tile([P, D], dt.float32, tag="o")
                    for nd in range(2):
                        ps = pmm.tile([P, S], dt.float32, tag="mm")
                        for ck in range(NPAIR):
                            nc.tensor.matmul(
                                ps[:],
                                attn_sb[:, ck, tt * P:(tt + 1) * P],
                                wout_sb[:, ck, nd * S:(nd + 1) * S],
                                start=(ck == 0),
                                stop=(ck == NPAIR - 1),
                            )
                        nc.vector.tensor_copy(o_sb[:, nd * S:(nd + 1) * S], ps[:])
                    nc.sync.dma_start(out_d[s, tt * P:(tt + 1) * P, :], o_sb[:])

    nc.finalize()
    return nc


_PROGRAM_CACHE: dict = {}


def _get_program(with_bias: bool):
    if with_bias not in _PROGRAM_CACHE:
        _PROGRAM_CACHE[with_bias] = build_program(with_bias)
    return _PROGRAM_CACHE[with_bias]


def make_in_maps(x, Wqkv, bqkv, Wout, with_bias):
    """Build the 8 per-core input maps from full inputs."""
    x = np.asarray(x, dtype=F32)
    # branch 0: contiguous segments; branch 1: even/odd strided segments
    x0 = np.ascontiguousarray(x.reshape(B, 16, S, D)).reshape(32, S, D)
    x1 = np.stack((x[:, 0::2], x[:, 1::2]), axis=1).reshape(32, S, D)

    w_eff = []
    b_eff = []
    for br in range(2):
        wq = np.asarray(Wqkv[br], dtype=F32).T.copy()  # [D, 3D]
        wq[:, :D] *= SCALE_Q
        w_eff.append(wq.astype(BF16))
        bq = np.asarray(bqkv[br], dtype=F32).copy()
        bq[:D] *= SCALE_Q
        b_eff.append(bq.reshape(1, 3 * D).astype(BF16))
    wo_eff = [
        (np.asarray(Wout[br], dtype=F32).T * W_MIX).astype(BF16) for br in range(2)
    ]

    in_maps = []
    for c in range(NCORE):
        br = 0 if c < 4 else 1
        segs = (x0 if br == 0 else x1)[(c % 4) * NSEG:(c % 4 + 1) * NSEG]
        xt = np.ascontiguousarray(segs.transpose(0, 2, 1)).astype(BF16)
        m = {"xt": xt, "wqkv": w_eff[br], "wout": wo_eff[br]}
        if with_bias:
            m["bqkv"] = b_eff[br]
        in_maps.append(m)
    return in_maps


def assemble_output(core_outs, bout):
    """Combine per-core [NSEG, S, D] outputs into the full [B, L, D] result."""
    y0 = np.concatenate([core_outs[c] for c in range(4)], axis=0)  # [32, S, D]
    y1 = np.concatenate([core_outs[c] for c in range(4, 8)], axis=0)
    y = np.ascontiguousarray(y0.reshape(B, L, D))
    y1 = y1.reshape(B, 2, L // 2, D)
    y[:, 0::2] += y1[:, 0]
    y[:, 1::2] += y1[:, 1]
    bconst = (np.asarray(bout[0], dtype=F32) + np.asarray(bout[1], dtype=F32)) * W_MIX
    if np.any(bconst):
        y += bconst
    return y


def kernel(x, Wqkv, bqkv, Wout, bout):
    with_bias = bool(np.any(np.asarray(bqkv)))
    nc = _get_program(with_bias)
    in_maps = make_in_maps(x, Wqkv, bqkv, Wout, with_bias)
    res = run_bass_kernel_spmd(nc, in_maps, core_ids=list(range(NCORE)))
    core_outs = [res.results[c]["out"] for c in range(NCORE)]
    return assemble_output(core_outs, bout)


# revision 9
# speedup vs baseline: 1.3586x; 1.3586x over previous
"""Dilated attention (DilatedAttentionOP) as a Bass/Tile SPMD kernel on 8 TRN2 NeuronCores.

Reference semantics (hardcoded; see problem reference):
  DIL=(1,2,4), SEG=(512,512,512), H=16, D=1024, B=2, L=8192.
  pairs truncate to [(br0,off0),(br1,off0),(br1,off1)]:
    - branch 0 (dilation 1): standard attention over contiguous 512-token segments,
      weights Wqkv[0]/Wout[0], covers every position.
    - branch 1 (dilation 2): attention over 512-token segments of the even-position
      subsequence (off=0) and odd-position subsequence (off=1), weights index 1.
  output = (out_branch0 + out_branch1_interleaved) / 3  (+ (bout0+bout1)/3).

Sharding: 64 independent (batch, segment) attention problems of 512 tokens each;
cores 0-3 take the 32 branch-0 segments, cores 4-7 the 32 branch-1 segments
(8 segments/core, identical SPMD program; per-branch weights replicated to the
core's in_map).

Per-core math (bf16 matmuls, fp32 PSUM accumulate):
  xT[seg] [D=1024, 512] -> QT,KT [128ch x 512]x16 tiles (channels on partitions,
  Q pre-scaled by 1/8 folded into Wq), V natural [512tok, 1024ch].
  scoresT[k,q] per head via row-paired (tile_position) K=64 matmuls;
  exp on ACT; AVT pair-packed via col-paired matmuls; softmax denominator via
  ones[128,64]-lhsT matmuls giving denom broadcast across partitions; DVE
  reciprocal+multiply; out-projection back to natural [512tok, D] layout
  (Wout pre-scaled by 1/3).
"""

import numpy as np
import ml_dtypes

from concourse import bacc, mybir
from concourse import tile
from concourse.bass_utils import run_bass_kernel_spmd

BF16 = ml_dtypes.bfloat16
F32 = np.float32

# Problem constants (hardcoded per contract)
B, L, D, H = 2, 8192, 1024, 16
HD = D // H            # 64
S = 512                # segment length
NSEG = 8               # segments per core
NCORE = 8
P = 128
DK = D // P            # 8 contraction chunks over D
KT = S // P            # 4 k tiles per segment
NPAIR = H // 2         # 8 head pairs
SCALE_Q = 1.0 / np.sqrt(HD)  # 0.125
W_MIX = 1.0 / 3.0


def build_program(with_bias: bool, repeat: int = 1, stages: str = "abc",
                  fake_exp: bool = False):
    """repeat>1 re-executes the whole per-core pipeline that many times
    (overwriting the same outputs) — used only for differential timing.
    stages/fake_exp are timing-experiment knobs (wrong numerics)."""
    nc = bacc.Bacc(None, target_bir_lowering=False)
    dt = mybir.dt
    xt_d = nc.declare_dram_parameter("xt", [NSEG, D, S], dt.bfloat16, False)
    wqkv_d = nc.declare_dram_parameter("wqkv", [D, 3 * D], dt.bfloat16, False)
    wout_d = nc.declare_dram_parameter("wout", [D, D], dt.bfloat16, False)
    if with_bias:
        bq_d = nc.declare_dram_parameter("bqkv", [1, 3 * D], dt.bfloat16, False)
    out_d = nc.declare_dram_parameter("out", [NSEG, S, D], dt.float32, True)

    with tile.TileContext(nc) as tc:
        with (
            tc.tile_pool(name="wpool", bufs=1) as wpool,
            tc.tile_pool(name="xpool", bufs=2) as xpool,
            tc.tile_pool(name="qkpool", bufs=2) as qkpool,
            tc.tile_pool(name="vpool", bufs=2) as vpool,
            tc.tile_pool(name="epool", bufs=6) as epool,
            tc.tile_pool(name="apool", bufs=2) as apool,
            tc.tile_pool(name="rpool", bufs=2) as rpool,
            tc.tile_pool(name="opool", bufs=3) as opool,
            tc.tile_pool(name="pmm", bufs=2, space="PSUM") as pmm,
            tc.tile_pool(name="psc", bufs=2, space="PSUM") as psc,
            tc.tile_pool(name="pav", bufs=2, space="PSUM") as pav,
            tc.tile_pool(name="pdn", bufs=2, space="PSUM") as pdn,
        ):
            # Resident weights
            wqkv_sb = wpool.tile([P, DK, 3 * D], dt.bfloat16)
            nc.sync.dma_start(
                wqkv_sb[:], wqkv_d.rearrange("(dk p) c -> p dk c", p=P)
            )
            wout_sb = wpool.tile([P, DK, D], dt.bfloat16)
            nc.sync.dma_start(
                wout_sb[:], wout_d.rearrange("(dk p) c -> p dk c", p=P)
            )
            ones64 = wpool.tile([P, HD], dt.bfloat16)
            nc.vector.memset(ones64[:], 1.0)
            if with_bias:
                bq_sb = wpool.tile([1, 3 * D], dt.bfloat16)
                nc.sync.dma_start(bq_sb[:], bq_d[:])
                ones_row = wpool.tile([1, S], dt.bfloat16)
                nc.vector.memset(ones_row[:], 1.0)
                ones_col = wpool.tile([1, P], dt.bfloat16)
                nc.vector.memset(ones_col[:], 1.0)

            for s in [s for _ in range(repeat) for s in range(NSEG)]:
                xt_sb = xpool.tile([P, DK, S], dt.bfloat16, tag="xt")
                nc.sync.dma_start(
                    xt_sb[:], xt_d[s].rearrange("(dk p) t -> p dk t", p=P)
                )

                # ---- Stage A: QKV projections ----
                # Q/K in transposed layout: psum[ch_tile(128), tok(512)]
                NQK = 2 * D // P  # 16 tiles: m<8 Q, m>=8 K
                qk_sb = qkpool.tile([P, NQK, S], dt.bfloat16, tag="qk")
                for m in range(NQK):
                    ps = pmm.tile([P, S], dt.float32, tag="mm")
                    for dk in range(DK):
                        nc.tensor.matmul(
                            ps[:],
                            wqkv_sb[:, dk, m * P:(m + 1) * P],
                            xt_sb[:, dk, :],
                            start=(dk == 0),
                            stop=(dk == DK - 1 and not with_bias),
                        )
                    if with_bias:
                        nc.tensor.matmul(
                            ps[:],
                            bq_sb[0:1, m * P:(m + 1) * P],
                            ones_row[0:1, :],
                            start=False,
                            stop=True,
                        )
                    nc.vector.tensor_copy(qk_sb[:, m, :], ps[:])

                # V in natural layout: psum[tok_tile(128), vch(512)]
                v_sb = vpool.tile([P, KT, D], dt.bfloat16, tag="v")
                for tt in range(KT):
                    for nv in range(2):
                        ps = pmm.tile([P, S], dt.float32, tag="mm")
                        for dk in range(DK):
                            nc.tensor.matmul(
                                ps[:],
                                xt_sb[:, dk, tt * P:(tt + 1) * P],
                                wqkv_sb[:, dk, 2 * D + nv * S:2 * D + (nv + 1) * S],
                                start=(dk == 0),
                                stop=(dk == DK - 1 and not with_bias),
                            )
                        if with_bias:
                            nc.tensor.matmul(
                                ps[:],
                                ones_col[0:1, :],
                                bq_sb[0:1, 2 * D + nv * S:2 * D + (nv + 1) * S],
                                start=False,
                                stop=True,
                            )
                        nc.scalar.copy(v_sb[:, tt, nv * S:(nv + 1) * S], ps[:])

                if stages == "a":
                    o_sb = opool.tile([P, D], dt.float32, tag="o")
                    nc.vector.tensor_copy(o_sb[:], v_sb[:, 0, :])
                    nc.sync.dma_start(out_d[s, 0:P, :], o_sb[:])
                    continue

                # ---- Stage B: attention per head pair ----
                attn_sb = apool.tile([P, NPAIR, S], dt.bfloat16, tag="attn")
                for pr in range(NPAIR):
                    av = pav.tile([P, S], dt.float32, tag="av")
                    dn = pdn.tile([P, S], dt.float32, tag="dn")
                    for kt in range(KT):
                        # scoresT = K_chunk @ Q^T for the two heads (row-paired)
                        s1 = psc.tile([P, S], dt.float32, tag="sc")
                        s2 = psc.tile([P, S], dt.float32, tag="sc")
                        nc.tensor.matmul(
                            s1[:],
                            qk_sb[0:HD, NPAIR + pr, kt * P:(kt + 1) * P],
                            qk_sb[0:HD, pr, :],
                            start=True, stop=True,
                        )
                        nc.tensor.matmul(
                            s2[:],
                            qk_sb[HD:P, NPAIR + pr, kt * P:(kt + 1) * P],
                            qk_sb[HD:P, pr, :],
                            start=True, stop=True,
                        )
                        e1 = epool.tile([P, S], dt.bfloat16, tag="e")
                        e2 = epool.tile([P, S], dt.bfloat16, tag="e")
                        if fake_exp:
                            nc.scalar.copy(e1[:], s1[:])
                            nc.scalar.copy(e2[:], s2[:])
                        else:
                            nc.scalar.activation(
                                e1[:], s1[:], mybir.ActivationFunctionType.Exp
                            )
                            nc.scalar.activation(
                                e2[:], s2[:], mybir.ActivationFunctionType.Exp
                            )
                        # AVT accumulate, col-paired: h1 -> rows 0:64, h2 -> 64:128
                        nc.tensor.matmul(
                            av[0:HD, :],
                            v_sb[:, kt, (2 * pr) * HD:(2 * pr + 1) * HD],
                            e1[:],
                            start=(kt == 0), stop=(kt == KT - 1),
                        )
                        nc.tensor.matmul(
                            av[HD:P, :],
                            v_sb[:, kt, (2 * pr + 1) * HD:(2 * pr + 2) * HD],
                            e2[:],
                            start=(kt == 0), stop=(kt == KT - 1),
                        )
                        # denominators, broadcast over 64 partitions each
                        nc.tensor.matmul(
                            dn[0:HD, :], ones64[:], e1[:],
                            start=(kt == 0), stop=(kt == KT - 1),
                        )
                        nc.tensor.matmul(
                            dn[HD:P, :], ones64[:], e2[:],
                            start=(kt == 0), stop=(kt == KT - 1),
                        )
                    rcp = rpool.tile([P, S], dt.float32, tag="rcp")
                    nc.vector.reciprocal(rcp[:], dn[:])
                    nc.vector.tensor_mul(attn_sb[:, pr, :], av[:], rcp[:])

                if stages == "ab":
                    o_sb = opool.tile([P, D], dt.float32, tag="o")
                    nc.vector.tensor_copy(o_sb[:, 0:NPAIR * S // 8], attn_sb[:, 0, :])
                    nc.sync.dma_start(out_d[s, 0:P, :], o_sb[:])
                    continue

                # ---- Stage C: output projection (natural layout) ----
                for tt in range(KT):
                    o_sb = opool.tile([P, D], dt.float32, tag="o")
                    for nd in range(2):
                        ps = pmm.tile([P, S], dt.float32, tag="mm")
                        for ck in range(NPAIR):
                            nc.tensor.matmul(
                                ps[:],
                                attn_sb[:, ck, tt * P:(tt + 1) * P],
                                wout_sb[:, ck, nd * S:(nd + 1) * S],
                                start=(ck == 0),
                                stop=(ck == NPAIR - 1),
                            )
                        nc.vector.tensor_copy(o_sb[:, nd * S:(nd + 1) * S], ps[:])
                    nc.sync.dma_start(out_d[s, tt * P:(tt + 1) * P, :], o_sb[:])

    nc.finalize()
    return nc


_PROGRAM_CACHE: dict = {}


def _get_program(with_bias: bool):
    if with_bias not in _PROGRAM_CACHE:
        _PROGRAM_CACHE[with_bias] = build_program(with_bias)
    return _PROGRAM_CACHE[with_bias]


def make_in_maps(x, Wqkv, bqkv, Wout, with_bias):
    """Build the 8 per-core input maps from full inputs."""
    x = np.asarray(x, dtype=F32)
    # branch 0: contiguous segments; branch 1: even/odd strided segments
    x0 = np.ascontiguousarray(x.reshape(B, 16, S, D)).reshape(32, S, D)
    x1 = np.stack((x[:, 0::2], x[:, 1::2]), axis=1).reshape(32, S, D)

    w_eff = []
    b_eff = []
    for br in range(2):
        wq = np.asarray(Wqkv[br], dtype=F32).T.copy()  # [D, 3D]
        wq[:, :D] *= SCALE_Q
        w_eff.append(wq.astype(BF16))
        bq = np.asarray(bqkv[br], dtype=F32).copy()
        bq[:D] *= SCALE_Q
        b_eff.append(bq.reshape(1, 3 * D).astype(BF16))
    wo_eff = [
        (np.asarray(Wout[br], dtype=F32).T * W_MIX).astype(BF16) for br in range(2)
    ]

    in_maps = []
    for c in range(NCORE):
        br = 0 if c < 4 else 1
        segs = (x0 if br == 0 else x1)[(c % 4) * NSEG:(c % 4 + 1) * NSEG]
        xt = np.ascontiguousarray(segs.transpose(0, 2, 1)).astype(BF16)
        m = {"xt": xt, "wqkv": w_eff[br], "wout": wo_eff[br]}
        if with_bias:
            m["bqkv"] = b_eff[br]
        in_maps.append(m)
    return in_maps


def assemble_output(core_outs, bout):
    """Combine per-core [NSEG, S, D] outputs into the full [B, L, D] result."""
    y0 = np.concatenate([core_outs[c] for c in range(4)], axis=0)  # [32, S, D]
    y1 = np.concatenate([core_outs[c] for c in range(4, 8)], axis=0)
    y = np.ascontiguousarray(y0.reshape(B, L, D))
    y1 = y1.reshape(B, 2, L // 2, D)
    y[:, 0::2] += y1[:, 0]
    y[:, 1::2] += y1[:, 1]
    bconst = (np.asarray(bout[0], dtype=F32) + np.asarray(bout[1], dtype=F32)) * W_MIX
    if np.any(bconst):
        y += bconst
    return y


def kernel(x, Wqkv, bqkv, Wout, bout):
    with_bias = bool(np.any(np.asarray(bqkv)))
    nc = _get_program(with_bias)
    in_maps = make_in_maps(x, Wqkv, bqkv, Wout, with_bias)
    res = run_bass_kernel_spmd(nc, in_maps, core_ids=list(range(NCORE)))
    core_outs = [res.results[c]["out"] for c in range(NCORE)]
    return assemble_output(core_outs, bout)
